# revision 1
# baseline (speedup 1.0000x reference)
"""Trainium2 Bass kernel for nn_BmmEnsemble (species-routed CELU-MLP ensemble).

Strategy (data-parallel over atoms, 8 NeuronCores):
  host: stable-sort atoms by species, shard each species block across the 8
        cores (capacity C atoms/species/core, zero-padded), pre-transpose aev
        to feature-major bf16, pre-pack weights.
  device (per core, SPMD): for each (species s, atom-tile t of T columns):
        L1: z1 = W1^T x           (bf16 matmuls, 10 M-chunks x 3 K-chunks)
        celu via the (r, u) split:  celu(z) = r + 0.1*u - 0.1,
              r = max(z+b, 0),  u = exp(10*min(z+b, 0))
        so L2/L3 contract against [r; u] with weights [W; 0.1W] and bias
        b_eff = b - 0.1*sum_fin(W) folded on host.
        L3 output is only needed as a per-feature SUM over atoms: ACT
        accum_out produces running sums of r3 and u3; no 4th matmul and no
        per-atom output.
  host finish: c3 = sum_t(acc_r + 0.1*acc_u) - 0.1*C, energy = W4 . c3 in
        f64, subtract zero-pad contributions, add b4 terms, divide by 8
        (ensemble mean).
"""
import numpy as np
import ml_dtypes

BF16 = ml_dtypes.bfloat16

S = 4            # species
M = 8            # ensemble models
F0, F1, F2, F3 = 384, 160, 128, 96
ALPHA = 0.1
N_CORES = 8
T = 512          # atoms per tile = one PSUM bank of f32 (matmul free-dim max)

_BUILD_CACHE = {}


# ----------------------------------------------------------------------------
# device kernel builder
# ----------------------------------------------------------------------------
def build_kernel(C):
    """Build (and cache) the compiled Bacc module for per-species-per-core
    capacity C (multiple of T)."""
    if C in _BUILD_CACHE:
        return _BUILD_CACHE[C]

    import concourse.bacc as bacc
    import concourse.tile as tile
    import concourse.mybir as mybir

    NT = C // T
    NUNIT = S * NT
    ACC_COLS = NUNIT * M
    F32 = mybir.dt.float32
    F32R = mybir.dt.float32r
    DBF = mybir.dt.bfloat16
    ADD, MAX, MIN = mybir.AluOpType.add, mybir.AluOpType.max, mybir.AluOpType.min
    RELU = mybir.ActivationFunctionType.Relu
    EXP = mybir.ActivationFunctionType.Exp
    BANK = 512  # f32 elements per PSUM bank

    nc = bacc.Bacc("TRN2", target_bir_lowering=False, debug=False)

    aev_d = nc.dram_tensor("aevT", [S, NT, 128, 3 * T], DBF, kind="ExternalInput").ap()
    w1_d = nc.dram_tensor("w1", [S, 128, 3 * 1280], DBF, kind="ExternalInput").ap()
    w2m_d = nc.dram_tensor("w2m", [S, 128, 2048], F32R, kind="ExternalInput").ap()
    w2r_d = nc.dram_tensor("w2r", [S, 128, 512], F32R, kind="ExternalInput").ap()
    w3_d = nc.dram_tensor("w3", [S, 128, 1536], F32R, kind="ExternalInput").ap()
    bl1_d = nc.dram_tensor("bl1", [S, 128, 384], F32R, kind="ExternalInput").ap()
    bl2_d = nc.dram_tensor("bl2", [S, 128, 256], F32R, kind="ExternalInput").ap()
    bl3_d = nc.dram_tensor("bl3", [S, 128, 192], F32R, kind="ExternalInput").ap()
    ones_d = nc.dram_tensor("ones", [128, T], F32R, kind="ExternalInput").ap()
    accr_d = nc.dram_tensor("accr", [128, ACC_COLS], F32, kind="ExternalOutput").ap()
    accu_d = nc.dram_tensor("accu", [128, ACC_COLS], F32, kind="ExternalOutput").ap()

    with tile.TileContext(nc) as tc:
        with tc.tile_pool(name="wpool", bufs=1) as wpool, \
             tc.tile_pool(name="w1pool", bufs=2) as w1pool, \
             tc.tile_pool(name="xpool", bufs=2) as xpool, \
             tc.tile_pool(name="h1pool", bufs=1) as h1pool, \
             tc.tile_pool(name="h2pool", bufs=1) as h2pool, \
             tc.tile_pool(name="upool", bufs=4) as upool, \
             tc.tile_pool(name="s3pool", bufs=2) as s3pool, \
             tc.tile_pool(name="apool", bufs=1) as apool, \
             tc.tile_pool(name="ps", bufs=2, space="PSUM") as psp:

            # --- per-species weights, DMA'd just-in-time inside the loop ---
            w2m_t, w2r_t, w3_t, bl1_t, bl2_t, bl3_t = [], [], [], [], [], []
            for s in range(S):
                w2m_t.append(wpool.tile([128, 2048], F32R, tag=f"w2m_{s}", name=f"w2m_{s}"))
                w2r_t.append(wpool.tile([128, 512], F32R, tag=f"w2r_{s}", name=f"w2r_{s}"))
                w3_t.append(wpool.tile([128, 1536], F32R, tag=f"w3_{s}", name=f"w3_{s}"))
                bl1_t.append(wpool.tile([128, 384], F32R, tag=f"bl1_{s}", name=f"bl1_{s}"))
                bl2_t.append(wpool.tile([128, 256], F32R, tag=f"bl2_{s}", name=f"bl2_{s}"))
                bl3_t.append(wpool.tile([128, 192], F32R, tag=f"bl3_{s}", name=f"bl3_{s}"))

            ones_t = wpool.tile([128, T], F32R, tag="ones", name="ones")
            nc.sync.dma_start(ones_t[:], ones_d)

            acc_r = apool.tile([128, ACC_COLS], F32, tag="acc_r", name="acc_r")
            acc_u = apool.tile([128, ACC_COLS], F32, tag="acc_u", name="acc_u")
            nc.vector.memset(acc_r[:], 0.0)
            nc.vector.memset(acc_u[:], 0.0)

            def group_view(ps_t, nrow, gsz):
                # strided view [nrow, gsz, T] over bank-aligned chunks
                return ps_t[0:nrow, :].rearrange("p (g q) -> p g q", q=BANK)[:, 0:gsz, 0:T]

            def bias_mms(ps_t, bl_t, nrow, gsz, c0, mwid):
                # chunk c0+g gets bias row at partition 32*((c0+g)%4), col block (c0+g)//4
                for g in range(gsz):
                    c = c0 + g
                    blk, j = c // 4, c % 4
                    nc.tensor.matmul(
                        ps_t[0:nrow, g * BANK:g * BANK + T],
                        bl_t[32 * j:32 * j + 1, blk * mwid:blk * mwid + nrow],
                        ones_t[32 * j:32 * j + 1, 0:T],
                        start=False, stop=True, tile_position=(32 * j, 0))

            # batched drains over one psum group (chunks are bias-complete)
            def drains(ps_t, nrow, gsz, r_view, u_view, r_on_act):
                ps_v = group_view(ps_t, nrow, gsz)
                ur = upool.tile([128, 2 * T], F32, tag="uscr", name="uscr")
                ur_v = ur[0:nrow, 0:gsz * T].rearrange("p (g q) -> p g q", q=T)
                nc.scalar.activation(ur_v, ps_v, EXP, bias=0.0, scale=10.0)
                nc.gpsimd.tensor_scalar(u_view, ur_v, 1.0, None, op0=MIN)
                if r_on_act:
                    nc.scalar.activation(r_view, ps_v, RELU, bias=0.0, scale=1.0)
                else:
                    nc.vector.tensor_scalar(r_view, ps_v, 0.0, None, op0=MAX)
                return ur

            pending_l3 = [None]

            def emit_l3(s, unit, r2, u2):
                for grp in range(4):
                    ps_t = psp.tile([96, 2 * BANK], F32, tag="psg", name="psg3", bufs=4)
                    for g in range(2):
                        m = 2 * grp + g
                        sl = slice(g * BANK, g * BANK + T)
                        nc.tensor.matmul(ps_t[:, sl],
                                         w3_t[s][:, (m * 2 + 0) * 96:(m * 2 + 1) * 96],
                                         r2[:, m * T:(m + 1) * T], start=True, stop=False)
                        nc.tensor.matmul(ps_t[:, sl],
                                         w3_t[s][:, (m * 2 + 1) * 96:(m * 2 + 2) * 96],
                                         u2[:, m * T:(m + 1) * T], start=False, stop=False)
                    bias_mms(ps_t, bl3_t[s], 96, 2, 2 * grp, 96)
                    ps_v = group_view(ps_t, 96, 2)
                    ur = upool.tile([128, 2 * T], F32, tag="uscr", name="uscr3")
                    ur_v = ur[0:96, 0:2 * T].rearrange("p (g q) -> p g q", q=T)
                    nc.scalar.activation(ur_v, ps_v, EXP, bias=0.0, scale=10.0)
                    for g in range(2):
                        m = 2 * grp + g
                        col = unit * M + m
                        r3 = s3pool.tile([96, T], F32, tag="r3scr", name="r3")
                        u3 = s3pool.tile([96, T], F32, tag="u3scr", name="u3")
                        nc.vector.tensor_scalar(
                            r3[:], ps_t[0:96, g * BANK:g * BANK + T],
                            0.0, None, op0=MAX, op1=ADD,
                            accum_out=acc_r[0:96, col:col + 1])
                        nc.vector.tensor_scalar(
                            u3[:], ur[0:96, g * T:(g + 1) * T], 1.0, None,
                            op0=MIN, op1=ADD,
                            accum_out=acc_u[0:96, col:col + 1])

            for s in range(S):
                w1s = w1pool.tile([128, 3 * 1280], DBF, tag="w1", name=f"w1s_{s}")
                nc.sync.dma_start(w1s[:, 0:1280], w1_d[s][:, 0:1280])
                x0_t = xpool.tile([128, 3 * T], DBF, tag="x", name="x_t")
                nc.sync.dma_start(x0_t[:], aev_d[s, 0])
                for k in range(1, 3):
                    nc.sync.dma_start(w1s[:, 1280 * k:1280 * (k + 1)],
                                      w1_d[s][:, 1280 * k:1280 * (k + 1)])
                nc.sync.dma_start(bl1_t[s][:], bl1_d[s])
                nc.sync.dma_start(w2m_t[s][:], w2m_d[s])
                nc.sync.dma_start(w2r_t[s][:], w2r_d[s])
                nc.sync.dma_start(bl2_t[s][:], bl2_d[s])
                nc.sync.dma_start(w3_t[s][:], w3_d[s])
                nc.sync.dma_start(bl3_t[s][:], bl3_d[s])
                for t in range(NT):
                    unit = s * NT + t
                    if t == 0:
                        x_t = x0_t
                    else:
                        x_t = xpool.tile([128, 3 * T], DBF, tag="x", name="x_t")
                        nc.sync.dma_start(x_t[:], aev_d[s, t])

                    # ---- L1: 10 M-chunks in groups of [4, 4, 2]
                    r1 = h1pool.tile([128, 10 * T], F32R, tag="r1", name="r1")
                    u1 = h1pool.tile([128, 10 * T], F32R, tag="u1", name="u1")
                    for c0, gsz in ((0, 2), (2, 2), (4, 2), (6, 2), (8, 2)):
                        ps_t = psp.tile([128, 2 * BANK], F32, tag="psg", name="psg", bufs=4)
                        for g in range(gsz):
                            c = c0 + g
                            for k in range(3):
                                nc.tensor.matmul(
                                    ps_t[:, g * BANK:g * BANK + T],
                                    w1s[:, 1280 * k + 128 * c:1280 * k + 128 * (c + 1)],
                                    x_t[:, k * T:(k + 1) * T],
                                    start=(k == 0), stop=False)
                        bias_mms(ps_t, bl1_t[s], 128, gsz, c0, 128)
                        rv = r1[:, c0 * T:(c0 + gsz) * T].rearrange("p (g q) -> p g q", q=T)
                        uv = u1[:, c0 * T:(c0 + gsz) * T].rearrange("p (g q) -> p g q", q=T)
                        drains(ps_t, 128, gsz, rv, uv, r_on_act=(c0 == 0))

                    if pending_l3[0] is not None:
                        emit_l3(*pending_l3[0])
                        pending_l3[0] = None

                    # ---- L2: 8 models in 2 groups of 4 (group == rem region)
                    r2 = h2pool.tile([128, 8 * T], F32R, tag="r2", name="r2")
                    u2 = h2pool.tile([128, 8 * T], F32R, tag="u2", name="u2")
                    for half in range(4):
                        m0 = 2 * half
                        reg = m0 // 4
                        ps_t = psp.tile([128, 2 * BANK], F32, tag="psg", name="psg", bufs=4)
                        for g in range(2):
                            m = m0 + g
                            sl = slice(g * BANK, g * BANK + T)
                            nc.tensor.matmul(ps_t[:, sl],
                                             w2m_t[s][:, (m * 2 + 0) * 128:(m * 2 + 1) * 128],
                                             r1[:, m * T:(m + 1) * T], start=True, stop=False)
                            nc.tensor.matmul(ps_t[:, sl],
                                             w2m_t[s][:, (m * 2 + 1) * 128:(m * 2 + 2) * 128],
                                             u1[:, m * T:(m + 1) * T], start=False, stop=False)
                            for ru in range(2):
                                h = (r1, u1)[ru]
                                j = m % 4
                                nc.tensor.matmul(
                                    ps_t[:, sl],
                                    w2r_t[s][32 * j:32 * (j + 1), (ru * 2 + reg) * 128:(ru * 2 + reg + 1) * 128],
                                    h[32 * j:32 * (j + 1), (8 + reg) * T:(9 + reg) * T],
                                    start=False, stop=False, tile_position=(32 * j, 0))
                        bias_mms(ps_t, bl2_t[s], 128, 2, m0, 128)
                        rv = r2[:, m0 * T:(m0 + 2) * T].rearrange("p (g q) -> p g q", q=T)
                        uv = u2[:, m0 * T:(m0 + 2) * T].rearrange("p (g q) -> p g q", q=T)
                        drains(ps_t, 128, 2, rv, uv, r_on_act=(half == 0))

                    pending_l3[0] = (s, unit, r2, u2)

            if pending_l3[0] is not None:
                emit_l3(*pending_l3[0])
                pending_l3[0] = None

            nc.sync.dma_start(accr_d, acc_r[:])
            nc.sync.dma_start(accu_d, acc_u[:])

    nc.compile()
    _BUILD_CACHE[C] = nc
    return nc


# ----------------------------------------------------------------------------
# host-side packing
# ----------------------------------------------------------------------------
def _celu64(x):
    return np.where(x > 0, x, ALPHA * np.expm1(np.minimum(x, 0) / ALPHA))


def prep_inputs(species, aev, W1, b1, W2, b2, W3, b3, W4, b4):
    """Returns (C, in_maps, finish) where finish(results) -> np scalar."""
    sp = np.asarray(species).reshape(-1)
    n_atoms = sp.shape[0]
    aev0 = np.asarray(aev, dtype=np.float32).reshape(n_atoms, F0)
    W1, b1, W2, b2, W3, b3, W4, b4 = [np.asarray(a, np.float64) for a in
                                      (W1, b1, W2, b2, W3, b3, W4, b4)]

    order = np.argsort(sp, kind="stable")
    cnt = np.bincount(sp.astype(np.int64), minlength=S)
    starts = np.concatenate([[0], np.cumsum(cnt)])
    # device capacity: floor to tile grid; the (small) per-species overflow is
    # evaluated on the host in f64 below
    C = max(T, (int(cnt.max()) // (N_CORES * T)) * T)
    NT = C // T
    dev_cnt = np.minimum(cnt, N_CORES * C)

    # per-core feature-major aev: [S, NT, 128, 3T] bf16
    aevTs = [np.zeros((S, NT, 128, 3 * T), dtype=BF16) for _ in range(N_CORES)]
    for s in range(S):
        block = aev0[order[starts[s]:starts[s] + dev_cnt[s]]]
        for c in range(N_CORES):
            seg = block[c * C:(c + 1) * C]
            if seg.shape[0] == 0:
                continue
            buf = np.zeros((C, F0), np.float32)
            buf[:seg.shape[0]] = seg
            # [C, 384] -> [NT, 128, 3T]:  [t, p, k*T+j] = buf[t*T+j, 128k+p]
            arr = buf.reshape(NT, T, 3, 128).transpose(0, 3, 2, 1).reshape(NT, 128, 3 * T)
            aevTs[c][s] = arr.astype(BF16)

    # ---- weights ----
    w1p = np.zeros((S, 128, 3 * 1280), dtype=BF16)
    b1c = np.zeros((S, 128, 10), np.float32)   # per-chunk bias columns
    for s in range(S):
        cols = np.zeros((F0, 1280), np.float64)
        for c in range(8):
            cols[:, 128 * c:128 * (c + 1)] = W1[s, c, :, 0:128]
            b1c[s, :, c] = b1[s, c, 0, 0:128]
        for r in range(2):
            for j in range(4):
                cols[:, 1024 + 128 * r + 32 * j:1024 + 128 * r + 32 * (j + 1)] = \
                    W1[s, 4 * r + j, :, 128:160]
                b1c[s, 32 * j:32 * (j + 1), 8 + r] = b1[s, 4 * r + j, 0, 128:160]
        w1p[s] = cols.reshape(3, 128, 1280).transpose(1, 0, 2).reshape(128, 3840).astype(BF16)

    w2m = np.zeros((S, 128, 2048), np.float32)
    w2r = np.zeros((S, 128, 512), np.float32)
    b2c = np.zeros((S, 128, 8), np.float32)
    for s in range(S):
        for m in range(M):
            reg, j = m // 4, m % 4
            for ru, sc in ((0, 1.0), (1, ALPHA)):
                w2m[s, :, (m * 2 + ru) * 128:(m * 2 + ru + 1) * 128] = sc * W2[s, m, 0:128, :]
                w2r[s, 32 * j:32 * (j + 1), (ru * 2 + reg) * 128:(ru * 2 + reg + 1) * 128] = \
                    sc * W2[s, m, 128:160, :]
            b2c[s, :, m] = (b2[s, m, 0, :] - ALPHA * W2[s, m].sum(axis=0)).astype(np.float32)

    w3p = np.zeros((S, 128, 1536), np.float32)
    b3c = np.zeros((S, 128, 8), np.float32)
    for s in range(S):
        for m in range(M):
            for ru, sc in ((0, 1.0), (1, ALPHA)):
                w3p[s, :, (m * 2 + ru) * 96:(m * 2 + ru + 1) * 96] = sc * W3[s, m, :, :]
            b3c[s, 0:96, m] = (b3[s, m, 0, :] - ALPHA * W3[s, m].sum(axis=0)).astype(np.float32)

    # bias-row lhsT tensors: chunk c -> partition 32*(c%4), col block c//4
    bl1 = np.zeros((S, 128, 384), np.float32)
    for s in range(S):
        for c in range(10):
            bl1[s, 32 * (c % 4), (c // 4) * 128:(c // 4) * 128 + 128] = b1c[s, :, c]
    bl2 = np.zeros((S, 128, 256), np.float32)
    for s in range(S):
        for c in range(8):
            bl2[s, 32 * (c % 4), (c // 4) * 128:(c // 4) * 128 + 128] = b2c[s, :, c]
    bl3 = np.zeros((S, 128, 192), np.float32)
    for s in range(S):
        for c in range(8):
            bl3[s, 32 * (c % 4), (c // 4) * 96:(c // 4) * 96 + 96] = b3c[s, 0:96, c]

    common = {"w1": w1p, "w2m": w2m, "w2r": w2r, "w3": w3p,
              "bl1": bl1, "bl2": bl2, "bl3": bl3,
              "ones": np.ones((128, T), np.float32)}
    in_maps = [dict(common, aevT=aevTs[c]) for c in range(N_CORES)]

    # ---- host finishing constants (f64) ----
    # zero-input chain value per species (pad correction), without b4
    e_pad = np.zeros(S)
    for s in range(S):
        h = _celu64(b1[s, :, 0, :])                       # [M, 160]
        h = _celu64(np.einsum("mf,mfo->mo", h, W2[s]) + b2[s, :, 0, :])
        h = _celu64(np.einsum("mf,mfo->mo", h, W3[s]) + b3[s, :, 0, :])
        e_pad[s] = np.einsum("mf,mf->", h, W4[s, :, :, 0])
    b4sum = b4[:, :, 0, 0].sum(axis=1)                    # [S]

    # exact f64 evaluation of atoms that overflow the device tile grid
    leftover = 0.0
    for s in range(S):
        n_left = int(cnt[s] - dev_cnt[s])
        if n_left <= 0:
            continue
        idx = order[starts[s] + dev_cnt[s]:starts[s + 1]]
        x = aev0[idx].astype(np.float64)                   # [n_left, 384]
        for m in range(M):
            h = _celu64(x @ W1[s, m] + b1[s, m, 0])
            h = _celu64(h @ W2[s, m] + b2[s, m, 0])
            h = _celu64(h @ W3[s, m] + b3[s, m, 0])
            leftover += float((h @ W4[s, m, :, 0]).sum()) + n_left * float(b4[s, m, 0, 0])

    def finish(results):
        accr = np.zeros((128, S * NT * M), np.float64)
        accu = np.zeros_like(accr)
        for res in results:
            accr += res["accr"].astype(np.float64)
            accu += res["accu"].astype(np.float64)
        # c3[s, m, f] = sum over all N_CORES*C padded atoms of celu(z3)
        tot = 0.0
        for s in range(S):
            c3 = np.zeros((M, F3))
            for t in range(NT):
                cols = (s * NT + t) * M
                c3 += (accr[0:96, cols:cols + M] + ALPHA * accu[0:96, cols:cols + M]).T
            c3 -= ALPHA * C * N_CORES
            tot += np.einsum("mf,mf->", c3, W4[s, :, :, 0])
            n_pad = N_CORES * C - dev_cnt[s]
            tot -= n_pad * e_pad[s]
            tot += dev_cnt[s] * b4sum[s]
        tot += leftover
        return np.array([tot / M], dtype=np.float32)

    return C, in_maps, finish


def _ensure_axon_platform():
    """Best-effort: make sure jax's default platform exposes the 8 NeuronCores
    (run_bass_kernel_spmd dispatches via jax.devices())."""
    try:
        import jax
        devs = jax.devices()
        if len(devs) >= N_CORES and devs[0].platform != "cpu":
            return
        jax.config.update("jax_platforms", "axon")
    except Exception:
        pass


def kernel(**inputs):
    from concourse.bass_utils import run_bass_kernel_spmd
    _ensure_axon_platform()
    C, in_maps, finish = prep_inputs(**inputs)
    nc = build_kernel(C)
    res = run_bass_kernel_spmd(nc, in_maps, list(range(N_CORES)))
    return finish(res.results)



# revision 2
# speedup vs baseline: 1.0784x; 1.0784x over previous
"""Trainium2 Bass kernel for nn_BmmEnsemble (species-routed CELU-MLP ensemble).

v2 design (one species per core-pair, 8 NeuronCores):
  sharding: core = 2*species + atom_half; each core runs 6144 atoms of ONE
    species through all 8 ensemble models (3 units x 2048 atoms).
  math: celu(z)+0.1 == min(0.1*e^{10 z}, max(z+0.1, 0.1))  (exact; exp branch
    lies above the linear branch everywhere), so each layer drains in exactly
    two passes:
      ACT : t = Exp(scale*zpp + c)          (psum -> bf16)
      DVE : h = (t MIN k) MAX zpp           (scalar_tensor_tensor, psum -> bf16)
    where zpp = z + b + 0.1 arrives pre-biased in PSUM:
      L1: bias rows ride the fp8 DoubleRow k-tail (3-row fp8 split for accuracy)
      L2/L3: one f32r bias matmul per psum bank (ones vector x bias row)
  L1 is fp8e4 DoubleRow (2 rows/cycle); W1 scaled x16 to avoid fp8 subnormals,
  un-scaled by folding 1/16 into W2. L2/L3 are bf16 (weight noise averages out;
  biases stay exact via f32r).
  L3 psum is dense-packed (8 models x 96 feats = 6 x 128-partition chunks) and
  its stt drains carry accum_out: acc[p, (c,u)] = sum_atoms (celu3+0.1).
  Host finish: E = sum w4[p]*(acc - 0.1*n) + n*sum(b4), leftover atoms in f64.
"""
import numpy as np
import ml_dtypes

BF = ml_dtypes.bfloat16
F8 = ml_dtypes.float8_e4m3fn

S = 4
M = 8
F0, F1, F2, F3 = 384, 160, 128, 96
ALPHA = 0.1
N_CORES = 8
NU = 6            # units per core
TU = 1024         # atoms per unit
C_DEV = NU * TU   # 6144 device atoms per core
W1SCALE = 16.0
BANK = 512

# L3 dense packing: 8 models x 96 feats -> 6 chunks of 128 partitions.
# segments (chunk, model, f0, width, p0) with legal tile_position col offsets
# (w<=32 -> p0 in {0,32,64,96}; w<=64 -> {0,64}; else p0==0).
def _l3_segments():
    segs = []
    for m in range(M):
        start, end = 96 * m, 96 * m + 96
        while start < end:
            c, p0 = start // 128, start % 128
            w = min(end, (c + 1) * 128) - start
            # split illegal (p0=32, w=96) into (32,32)+(64,64)
            if p0 == 32 and w == 96:
                segs.append((c, m, start - 96 * m, 32, 32))
                segs.append((c, m, start - 96 * m + 32, 64, 64))
            else:
                segs.append((c, m, start - 96 * m, w, p0))
            start += w
    return segs


L3SEGS = _l3_segments()
# L3 drains converted to ACT-relu-accum + DVE min-accum (DVE -> ACT rebalance)
STYLE_B = set()
_BUILD_CACHE = {}


def build_kernel():
    if "nc" in _BUILD_CACHE:
        return _BUILD_CACHE["nc"]
    import concourse.bacc as bacc
    import concourse.tile as tile
    import concourse.mybir as mybir

    F32 = mybir.dt.float32
    F32R = mybir.dt.float32r
    DBF = mybir.dt.bfloat16
    DF8 = mybir.dt.float8e4
    MIN, MAX = mybir.AluOpType.min, mybir.AluOpType.max
    MULT = mybir.AluOpType.mult
    EXP = mybir.ActivationFunctionType.Exp
    RELU = mybir.ActivationFunctionType.Relu
    DR = mybir.MatmulPerfMode.DoubleRow

    nc = bacc.Bacc("TRN2", target_bir_lowering=False, debug=False)

    aevA_d = nc.dram_tensor("aevA", [NU, 128, 2 * TU], DF8, kind="ExternalInput").ap()
    aevB_d = nc.dram_tensor("aevB", [NU, 67, 2 * TU], DF8, kind="ExternalInput").ap()
    w1a_d = nc.dram_tensor("w1a", [128, 10 * 256], DF8, kind="ExternalInput").ap()
    w1b_d = nc.dram_tensor("w1b", [67, 10 * 256], DF8, kind="ExternalInput").ap()
    w2m_d = nc.dram_tensor("w2m", [128, M * 128], DBF, kind="ExternalInput").ap()
    w2r_d = nc.dram_tensor("w2r", [128, M * 128], DBF, kind="ExternalInput").ap()
    w2b_d = nc.dram_tensor("w2b", [128, M * 128], F32R, kind="ExternalInput").ap()
    w3s_d = nc.dram_tensor("w3s", [128, 14 * 96], DBF, kind="ExternalInput").ap()
    w3b_d = nc.dram_tensor("w3b", [1, 6 * 128], F32R, kind="ExternalInput").ap()
    zrow_d = nc.dram_tensor("zrow", [1, 128], F32R, kind="ExternalInput").ap()
    ones_d = nc.dram_tensor("ones", [128, BANK], F32R, kind="ExternalInput").ap()
    cexp_d = nc.dram_tensor("cexp", [128, 3], F32, kind="ExternalInput").ap()
    acc_d = nc.dram_tensor("acc", [128, NU * 12], F32, kind="ExternalOutput").ap()

    # w3 segment column offsets in w3s
    seg_off = []
    off = 0
    for (c, m, f0, w, p0) in L3SEGS:
        seg_off.append(off)
        off += w
    assert off <= 14 * 96

    with tile.TileContext(nc) as tc:
        with tc.tile_pool(name="wpool", bufs=1) as wpool, \
             tc.tile_pool(name="aevp", bufs=3) as aevp, \
             tc.tile_pool(name="h1p", bufs=2) as h1p, \
             tc.tile_pool(name="h2p", bufs=2) as h2p, \
             tc.tile_pool(name="tp", bufs=6) as tp, \
             tc.tile_pool(name="lp", bufs=6) as lp, \
             tc.tile_pool(name="h3p", bufs=3) as h3p, \
             tc.tile_pool(name="accp", bufs=1) as accp, \
             tc.tile_pool(name="ps", bufs=4, space="PSUM") as psp:

            w1a_t = wpool.tile([128, 10 * 256], DF8, tag="w1a", name="w1a")
            w1b_t = wpool.tile([128, 10 * 256], DF8, tag="w1b", name="w1b")
            w2m_t = wpool.tile([128, M * 128], DBF, tag="w2m", name="w2m")
            w2r_t = wpool.tile([128, M * 128], DBF, tag="w2r", name="w2r")
            w2b_t = wpool.tile([128, M * 128], F32R, tag="w2b", name="w2b")
            w3s_t = wpool.tile([128, 14 * 96], DBF, tag="w3s", name="w3s")
            w3b_t = wpool.tile([128, 6 * 128], F32R, tag="w3b", name="w3b")
            zrow_t = wpool.tile([128, 128], F32R, tag="zrow", name="zrow")
            ones_t = wpool.tile([128, BANK], F32R, tag="ones", name="ones")
            cexp_t = wpool.tile([128, 3], F32, tag="cexp", name="cexp")
            acc_t = accp.tile([128, NU * 12], F32, tag="acc", name="acc")

            nc.sync.dma_start(w1a_t[:], w1a_d)
            nc.sync.dma_start(w1b_t[0:67, :], w1b_d)
            nc.sync.dma_start(w2m_t[:], w2m_d)
            nc.sync.dma_start(w2r_t[:], w2r_d)
            nc.sync.dma_start(w2b_t[:], w2b_d)
            nc.sync.dma_start(w3s_t[:], w3s_d)
            nc.sync.dma_start(w3b_t[0:1, :], w3b_d)
            nc.sync.dma_start(zrow_t[0:1, :], zrow_d)
            nc.sync.dma_start(ones_t[:], ones_d)
            nc.sync.dma_start(cexp_t[:], cexp_d)
            c1_ap = cexp_t[:, 0:1]   # ln(1.6) - 1
            c2_ap = cexp_t[:, 1:2]   # ln(0.1) - 1
            c3_ap = cexp_t[:, 2:3]   # -0.1 (relu shift)

            for u in range(NU):
                aevA_t = aevp.tile([128, 2 * TU], DF8, tag="aevA", name="aevA")
                aevB_t = aevp.tile([128, 2 * TU], DF8, tag="aevB", name="aevB")
                nc.sync.dma_start(aevA_t[:], aevA_d[u])
                nc.sync.dma_start(aevB_t[0:67, :], aevB_d[u])
                aevA_v = aevA_t[:].rearrange("p (two n) -> p two n", two=2)
                aevB_v = aevB_t[0:67, :].rearrange("p (two n) -> p two n", two=2)

                # ---- L1: 10 chunks, fp8 DoubleRow; rem chunks (8,9) first so
                # L2 model 0 unblocks after chunk 0 drains.
                h1 = [None] * 10
                for c in [8, 9] + list(range(8)):
                    ps = psp.tile([128, TU], F32, tag="ps", name=f"psL1_{u}_{c}")
                    la = w1a_t[:, c * 256:(c + 1) * 256].rearrange(
                        "p (two f) -> p two f", two=2)
                    lb = w1b_t[0:67, c * 256:(c + 1) * 256].rearrange(
                        "p (two f) -> p two f", two=2)
                    for b in range(TU // BANK):
                        sl = slice(b * BANK, (b + 1) * BANK)
                        nc.tensor.matmul(ps[:, sl], la, aevA_v[:, :, sl],
                                         start=True, stop=False, perf_mode=DR)
                        nc.tensor.matmul(ps[:, sl], lb, aevB_v[:, :, sl],
                                         start=False, stop=True, perf_mode=DR)
                    t_t = tp.tile([128, TU], DBF, tag="t", name="t1")
                    nc.scalar.activation(t_t[:], ps[:], EXP,
                                         bias=c1_ap, scale=10.0 / W1SCALE)
                    h1[c] = h1p.tile([128, TU], DBF, tag=f"h1_{c}", name=f"h1_{c}")
                    nc.vector.scalar_tensor_tensor(
                        h1[c][:], t_t[:], 1.6, ps[:], op0=MIN, op1=MAX)

                # ---- L2: 8 models, bf16 main + rem + f32r bias matmuls
                h2 = [None] * M
                for m in range(M):
                    ps = psp.tile([128, TU], F32, tag="ps", name=f"psL2_{u}_{m}")
                    j, r = m % 4, m // 4
                    for b in range(TU // BANK):
                        sl = slice(b * BANK, (b + 1) * BANK)
                        nc.tensor.matmul(ps[:, sl],
                                         w2b_t[32 * j:32 * j + 1,
                                               m * 128:(m + 1) * 128],
                                         ones_t[32 * j:32 * j + 1, 0:BANK],
                                         start=True, stop=False,
                                         tile_position=(32 * j, 0))
                        nc.tensor.matmul(ps[:, sl],
                                         w2r_t[32 * j:32 * (j + 1),
                                               m * 128:(m + 1) * 128],
                                         h1[8 + r][32 * j:32 * (j + 1), sl],
                                         start=False, stop=False,
                                         tile_position=(32 * j, 0))
                        nc.tensor.matmul(ps[:, sl],
                                         w2m_t[:, m * 128:(m + 1) * 128],
                                         h1[m][:, sl], start=False, stop=True)
                    t_t = tp.tile([128, TU], DBF, tag="t", name="t2")
                    nc.scalar.activation(t_t[:], ps[:], EXP,
                                         bias=c2_ap, scale=10.0)
                    h2[m] = h2p.tile([128, TU], DBF, tag=f"h2_{m}", name=f"h2_{m}")
                    nc.vector.scalar_tensor_tensor(
                        h2[m][:], t_t[:], 0.1, ps[:], op0=MIN, op1=MAX)

                # ---- L3: 6 dense chunks; stt drains carry accum_out
                for c in range(6):
                    ps = psp.tile([128, TU], F32, tag="ps", name=f"psL3_{u}_{c}")
                    csegs = [(i, s) for i, s in enumerate(L3SEGS) if s[0] == c]
                    for b in range(TU // BANK):
                        sl = slice(b * BANK, (b + 1) * BANK)
                        nc.tensor.matmul(ps[:, sl],
                                         w3b_t[0:1, c * 128:(c + 1) * 128],
                                         ones_t[0:1, :], start=True, stop=False)
                        for k, (i, (cc, m, f0, w, p0)) in enumerate(csegs):
                            o = seg_off[i]
                            nc.tensor.matmul(
                                ps[p0:p0 + w, sl],
                                w3s_t[:, o:o + w],
                                h2[m][:, sl],
                                start=False, stop=False,
                                tile_position=(0, p0))
                        # full-partition closer (adds 0): sim group tracking
                        # is partition-offset-blind, so the stop must span 128
                        nc.tensor.matmul(ps[:, sl], zrow_t[0:1, :],
                                         ones_t[0:1, :], start=False, stop=True)
                    col = (u * 6 + c) * 2
                    t_t = tp.tile([128, TU], DBF, tag="t", name="t3")
                    nc.scalar.activation(t_t[:], ps[:], EXP,
                                         bias=c2_ap, scale=10.0)
                    h3 = h3p.tile([128, TU], DBF, tag="h3", name="h3")
                    if (u, c) in STYLE_B:
                        # ACT: acc_r = sum relu(z+b); DVE: acc_u = sum min(10*t3, 1)
                        nc.scalar.activation(h3[:], ps[:], RELU,
                                             bias=c3_ap, scale=1.0,
                                             accum_out=acc_t[:, col:col + 1])
                        u3 = h3p.tile([128, TU], DBF, tag="u3", name="u3")
                        nc.vector.tensor_scalar(
                            u3[:], t_t[:], 10.0, 1.0, op0=MULT, op1=MIN,
                            accum_out=acc_t[:, col + 1:col + 2])
                    else:
                        nc.vector.scalar_tensor_tensor(
                            h3[:], t_t[:], 0.1, ps[:], op0=MIN, op1=MAX,
                            accum_out=acc_t[:, col:col + 1])

            nc.sync.dma_start(acc_d, acc_t[:])

    nc.compile()
    _BUILD_CACHE["nc"] = nc
    return nc


# ----------------------------------------------------------------------------
# host-side packing
# ----------------------------------------------------------------------------
def _celu64(x):
    return np.where(x > 0, x, ALPHA * np.expm1(np.minimum(x, 0) / ALPHA))


def _fp8_3row(v):
    """split vector v into 3 fp8 rows summing to ~v (error ~(0.06)^3)."""
    hi = v.astype(F8)
    mid = (v - hi.astype(np.float64)).astype(F8)
    lo = (v - hi.astype(np.float64) - mid.astype(np.float64)).astype(F8)
    return hi, mid, lo


def prep_inputs(species, aev, W1, b1, W2, b2, W3, b3, W4, b4):
    sp = np.asarray(species).reshape(-1)
    n_atoms = sp.shape[0]
    aev0 = np.asarray(aev, dtype=np.float32).reshape(n_atoms, F0)
    W1, b1, W2, b2, W3, b3, W4, b4 = [np.asarray(a, np.float64) for a in
                                      (W1, b1, W2, b2, W3, b3, W4, b4)]

    order = np.argsort(sp, kind="stable")
    cnt = np.bincount(sp.astype(np.int64), minlength=S)
    starts = np.concatenate([[0], np.cumsum(cnt)])

    # per-core atom assignment: core = 2*s + half, capacity C_DEV each
    core_atoms = []       # aev rows per core [n_c, 384]
    leftover_idx = []     # per species: overflow atom indices
    for s in range(S):
        idx = order[starts[s]:starts[s + 1]]
        half = (len(idx) + 1) // 2
        halves = [idx[:half], idx[half:]]
        for h in range(2):
            take = halves[h][:C_DEV]
            core_atoms.append(take)
            leftover_idx.append(halves[h][C_DEV:])

    # ---- chunk mapping for L1: chunks 0..7 = model c feats 0:128;
    # chunk 8+r: col j -> model 4r + j//32, feat 128 + j%32
    def l1_cols(c):
        out = []
        if c < 8:
            for j in range(128):
                out.append((c, j))
        else:
            r = c - 8
            for j in range(128):
                out.append((4 * r + j // 32, 128 + j % 32))
        return out

    # ---- per-species weight packs (shared by the core pair) ----
    packs = []
    for s in range(S):
        w1a = np.zeros((128, 10 * 256), dtype=F8)
        w1b = np.zeros((67, 10 * 256), dtype=F8)
        for c in range(10):
            cols = l1_cols(c)
            wblk = np.zeros((384, 128), np.float64)
            brow = np.zeros(128, np.float64)
            for j, (m, f) in enumerate(cols):
                wblk[:, j] = W1[s, m, :, f] * W1SCALE
                brow[j] = (b1[s, m, 0, f] + ALPHA) * W1SCALE
            # w1a: partition p, block t: feat 128t + p
            w1a[:, c * 256:c * 256 + 128] = wblk[0:128].astype(F8)
            w1a[:, c * 256 + 128:(c + 1) * 256] = wblk[128:256].astype(F8)
            # w1b: parts 0:64 feats 256+64t+p; parts 64..66 = bias 3-row split
            w1b[0:64, c * 256:c * 256 + 128] = wblk[256:320].astype(F8)
            w1b[0:64, c * 256 + 128:(c + 1) * 256] = wblk[320:384].astype(F8)
            hi, mid, lo = _fp8_3row(brow)
            w1b[64, c * 256:c * 256 + 128] = hi
            w1b[65, c * 256:c * 256 + 128] = mid
            w1b[66, c * 256:c * 256 + 128] = lo

        w2m = np.zeros((128, M * 128), dtype=BF)
        w2r = np.zeros((128, M * 128), dtype=BF)
        w2b = np.zeros((128, M * 128), np.float32)
        for m in range(M):
            j = m % 4
            w2m[:, m * 128:(m + 1) * 128] = (W2[s, m, 0:128, :] / W1SCALE).astype(BF)
            w2r[32 * j:32 * (j + 1), m * 128:(m + 1) * 128] = \
                (W2[s, m, 128:160, :] / W1SCALE).astype(BF)
            # fold the +0.1 offset of h~1 = W1SCALE*(celu+0.1) using the ACTUAL
            # quantized device weights (coherent-error cancellation)
            w2q_sum = (w2m[:, m * 128:(m + 1) * 128].astype(np.float64).sum(axis=0)
                       + w2r[32 * j:32 * (j + 1),
                             m * 128:(m + 1) * 128].astype(np.float64).sum(axis=0))
            w2b[32 * (m % 4), m * 128:(m + 1) * 128] = \
                (b2[s, m, 0, :] - ALPHA * W1SCALE * w2q_sum + ALPHA)

        w3s = np.zeros((128, 14 * 96), dtype=BF)
        w3b = np.zeros((1, 6 * 128), np.float32)
        w3q_sum = np.zeros((M, 96))
        off = 0
        for (c, m, f0, w, p0) in L3SEGS:
            w3s[:, off:off + w] = W3[s, m, :, f0:f0 + w].astype(BF)
            w3q_sum[m, f0:f0 + w] = \
                w3s[:, off:off + w].astype(np.float64).sum(axis=0)
            off += w
        for c in range(6):
            for p in range(128):
                g = 128 * c + p
                m, f = g // 96, g % 96
                w3b[0, c * 128 + p] = \
                    (b3[s, m, 0, f] - ALPHA * w3q_sum[m, f] + ALPHA)
        packs.append(dict(w1a=w1a, w1b=w1b, w2m=w2m, w2r=w2r,
                          w2b=w2b.astype(np.float32), w3s=w3s,
                          w3b=w3b.astype(np.float32)))

    cexp = np.zeros((128, 3), np.float32)
    cexp[:, 0] = np.log(ALPHA * W1SCALE) - 1.0                 # ln(1.6) - 1
    cexp[:, 1] = np.log(ALPHA) - 1.0                           # ln(0.1) - 1
    cexp[:, 2] = -ALPHA
    ones = np.ones((128, BANK), np.float32)
    zrow = np.zeros((1, 128), np.float32)

    in_maps = []
    for core in range(N_CORES):
        s = core // 2
        idx = core_atoms[core]
        buf = np.zeros((C_DEV, F0), np.float32)
        buf[:len(idx)] = aev0[idx]
        x = buf.reshape(NU, TU, F0)
        aevA = np.zeros((NU, 128, 2 * TU), dtype=F8)
        aevB = np.zeros((NU, 67, 2 * TU), dtype=F8)
        for t in range(2):
            aevA[:, :, t * TU:(t + 1) * TU] = \
                x[:, :, 128 * t:128 * (t + 1)].transpose(0, 2, 1).astype(F8)
            aevB[:, 0:64, t * TU:(t + 1) * TU] = \
                x[:, :, 256 + 64 * t:256 + 64 * (t + 1)].transpose(0, 2, 1).astype(F8)
        aevB[:, 64, 0:TU] = np.float32(1.0).astype(F8)   # ones row (block 0)
        aevB[:, 65, 0:TU] = np.float32(1.0).astype(F8)
        aevB[:, 66, 0:TU] = np.float32(1.0).astype(F8)
        in_maps.append(dict(packs[s], aevA=aevA, aevB=aevB,
                            ones=ones, cexp=cexp, zrow=zrow))

    # ---- host finishing constants ----
    # per (s): w4vec[c, p] = W4[s, m(c,p), f(c,p), 0]
    w4vec = np.zeros((S, 6, 128))
    for s in range(S):
        for c in range(6):
            for p in range(128):
                g = 128 * c + p
                w4vec[s, c, p] = W4[s, g // 96, g % 96, 0]
    b4sum = b4[:, :, 0, 0].sum(axis=1)          # [S]
    w4sum = w4vec.sum(axis=(1, 2))              # [S] = sum_{m,f} W4

    # pad-atom contribution (aev = 0 rows, only when a core is short)
    e_pad = np.zeros(S)
    for s in range(S):
        h = _celu64(b1[s, :, 0, :])
        h = _celu64(np.einsum("mf,mfo->mo", h, W2[s]) + b2[s, :, 0, :])
        h = _celu64(np.einsum("mf,mfo->mo", h, W3[s]) + b3[s, :, 0, :])
        e_pad[s] = np.einsum("mf,mf->", h, W4[s, :, :, 0])

    # leftover atoms evaluated exactly in f64
    leftover = 0.0
    n_left = np.zeros(S, np.int64)
    for s in range(S):
        for h in range(2):
            li = leftover_idx[2 * s + h]
            if len(li) == 0:
                continue
            n_left[s] += len(li)
            xx = aev0[li].astype(np.float64)
            for m in range(M):
                hh = _celu64(xx @ W1[s, m] + b1[s, m, 0])
                hh = _celu64(hh @ W2[s, m] + b2[s, m, 0])
                hh = _celu64(hh @ W3[s, m] + b3[s, m, 0])
                leftover += float((hh @ W4[s, m, :, 0]).sum())
    n_core_real = [len(core_atoms[c]) for c in range(N_CORES)]

    def finish(results):
        tot = 0.0
        for core in range(N_CORES):
            s = core // 2
            acc = np.asarray(results[core]["acc"], np.float64)  # [128, NU*12]
            # combine cols -> [6, 128] summed over units
            a = np.zeros((6, 128))
            for u in range(NU):
                for c in range(6):
                    col = (u * 6 + c) * 2
                    if (u, c) in STYLE_B:
                        a[c] += acc[:, col] + ALPHA * acc[:, col + 1] + ALPHA * TU
                    else:
                        a[c] += acc[:, col]
            tot += float((w4vec[s] * a).sum())
            # remove the +0.1 offset for all C_DEV processed rows (includes pads)
            tot -= ALPHA * C_DEV * w4sum[s]
            # remove pad-atom energies, add back b4 for real atoms
            n_pad = C_DEV - n_core_real[core]
            tot -= n_pad * e_pad[s]
            tot += n_core_real[core] * b4sum[s]
        for s in range(S):
            tot += n_left[s] * b4sum[s]
        tot += leftover
        return np.array([tot / M], dtype=np.float32)

    return in_maps, finish


def _ensure_axon_platform():
    try:
        import jax
        devs = jax.devices()
        if len(devs) >= N_CORES and devs[0].platform != "cpu":
            return
        jax.config.update("jax_platforms", "axon")
    except Exception:
        pass


def kernel(**inputs):
    from concourse.bass_utils import run_bass_kernel_spmd
    _ensure_axon_platform()
    in_maps, finish = prep_inputs(**inputs)
    nc = build_kernel()
    res = run_bass_kernel_spmd(nc, in_maps, list(range(N_CORES)))
    return finish(res.results)
